# revision 9
# baseline (speedup 1.0000x reference)
"""CondMlp Trainium2 kernel (v3).

Math (reference):
    xp = x @ W_pre + b_pre                 # [B, NI, DH]
    c  = query @ W_emb + b_emb             # [B, NO, DH]
    A  = xp @ W1[:DH] + b1                 # [B, NI, DH]   (host precompute, tiny)
    C2 = c @ W1[DH:]                       # [B, NO, DH]   (host precompute, tiny)
    h[b,i,o,:] = A[b,i,:] + C2[b,o,:]
    out[b,i,o,:] = gelu(h) @ W2 + b2       # [B, NI, NO, DOUT]

Sharding: 8 cores, core k handles batch b = k//2, NI-half h = k%2 (128 rows).

Design (measured constants from traces/microbench):
  * The kernel is ACT+DVE-bound: per core the gelu (58us, ACT-only), the
    broadcast adds (50us, DVE tensor_scalar at 2x_1P, 197ns/[128,256]) and
    the PSUM drains (1x port-bound; ACT 1.97us / DVE 2.29us per [128,2048])
    must share two engines -> ~89us/engine balanced.
  * bf16 output stores (halves DMA) assembled/untransposed on host.
  * W2-stationary matmuls, N=512: back-to-back same-lhsT MMs run at stream
    rate (216ns, LDWEIGHTS hidden by the PE background weight buffer).
  * Drains split ACT/DVE ~15/17 (the LP balance point).
  * 8-row first/last groups + per-subgroup 512KB stores shorten the
    pipeline head/tail; PE warmup matmuls flip the HAM clock-gate early.
  * GPSIMD stock tensor_scalar measured 3.9us per [128,256] add (20x DVE):
    useless for compute; its SWDGE ring only carries the w2 loads.
"""

import numpy as np
import ml_dtypes

import concourse.bass as bass
import concourse.bacc as bacc
import concourse.mybir as mybir
from concourse.tile import TileContext
from concourse.bass_utils import run_bass_kernel_spmd

B, NI, NO = 4, 256, 256
DIN, DQ, DH, DOUT = 256, 256, 256, 256
NCORES = 8
RPC = (B * NI) // NCORES    # rows per core = 128
NSUB = RPC // 4             # 32 matmul subgroups of 4 rows
GROUP_ROWS = [8] + [16] * 7 + [8]   # taper head/tail
F32 = mybir.dt.float32
BF16 = mybir.dt.bfloat16

ACT_DRAINS = 15             # of 32 drains, how many go to ACT (rest DVE)

_nc_cache = None


def build_nc():
    nc = bacc.Bacc()

    c_t = nc.declare_dram_parameter("c_t", [DH, NO], BF16, isOutput=False)
    a_t = nc.declare_dram_parameter("a_t", [DH, RPC], F32, isOutput=False)
    w2 = nc.declare_dram_parameter("w2", [DH, DOUT], BF16, isOutput=False)
    # out[blk*2+d, p, (r, o)]: blk = 8-row block, d = dout chunk, p = dout
    # within chunk, free = r*256 + o. Host untransposes.
    out = nc.declare_dram_parameter("out", [NSUB, 128, 2048], BF16,
                                    isOutput=True)

    gelu = mybir.ActivationFunctionType.Gelu

    with TileContext(nc) as tc:
        with (
            tc.tile_pool(name="const", bufs=1) as cpool,
            tc.tile_pool(name="h", bufs=2) as hpool,
            tc.tile_pool(name="g", bufs=2) as gpool,
            tc.tile_pool(name="ps", bufs=2, space="PSUM") as pspool,
            tc.tile_pool(name="ostage", bufs=3) as opool,
        ):
            # Loads in gating order: the first adds need ct0+at0 only.
            ct, at, w2t = [], [], []
            for ch in range(2):
                t = cpool.tile([128, NO], BF16, tag=f"ct{ch}")
                ct.append(t)
                t = cpool.tile([128, RPC], F32, tag=f"at{ch}")
                at.append(t)
                t = cpool.tile([128, DOUT], BF16, tag=f"w2{ch}")
                w2t.append(t)
            nc.sync.dma_start(out=ct[0][:, :], in_=c_t[0:128, :])
            nc.sync.dma_start(out=at[0][:, :], in_=a_t[0:128, :])
            nc.sync.dma_start(out=ct[1][:, :], in_=c_t[128:256, :])
            nc.sync.dma_start(out=at[1][:, :], in_=a_t[128:256, :])
            for ch in range(2):
                nc.gpsimd.dma_start(out=w2t[ch][:, :],
                                    in_=w2[ch * 128:(ch + 1) * 128, :])

            # ACT warmup: pays the ~2.7us gelu table load during the ramp.
            scratch = cpool.tile([128, 2], F32, tag="scratch")
            nc.vector.memset(scratch[:, :], 0.0)
            nc.scalar.activation(scratch[:, :], scratch[:, :], gelu)

            # PE warmup: dummy matmuls flip the HAM clock-gate to 8/8
            # (2.4 GHz) before the first real matmul.
            dummy = cpool.tile([128, 128], BF16, tag="dummy")
            nc.vector.memset(dummy[:, :], 0.0)
            ps_w = pspool.tile([128, 2048], F32, tag="ps")
            for i in range(12):
                nc.tensor.matmul(out=ps_w[:, 0:128], lhsT=dummy[:, :],
                                 rhs=dummy[:, :], start=True, stop=True)

            drain_idx = 0
            row0 = 0
            for g, nrows in enumerate(GROUP_ROWS):
                # h/g free layout: (ch, r, o) -> ch*nrows*256 + r*256 + o
                h_buf = hpool.tile([128, nrows * 512], BF16, tag="h")
                g_buf = gpool.tile([128, nrows * 512], BF16, tag="g")

                for ch in range(2):
                    for r in range(nrows):
                        row = row0 + r
                        s = (ch * nrows + r) * 256
                        nc.vector.tensor_scalar_add(
                            out=h_buf[:, s:s + 256],
                            in0=ct[ch][:, :],
                            scalar1=at[ch][:, row:row + 1],
                        )
                    nc.scalar.activation(
                        g_buf[:, ch * nrows * 256:(ch + 1) * nrows * 256],
                        h_buf[:, ch * nrows * 256:(ch + 1) * nrows * 256], gelu)

                # 8-row matmul blocks: each stationary w2 slice is reused for
                # 4 back-to-back N=512 matmuls (stream rate, LDW hidden); 4
                # LDWEIGHTS per block instead of per-matmul thrash.
                for blk_i in range(nrows // 8):
                    blk = row0 // 8 + blk_i
                    rb = blk_i * 8
                    ps_d0 = pspool.tile([128, 2048], F32, tag="ps")
                    ps_d1 = pspool.tile([128, 2048], F32, tag="ps")
                    psd = (ps_d0, ps_d1)

                    def mm_phase(ch, d, stop):
                        for q in range(4):
                            r0 = rb + 2 * q
                            nc.tensor.matmul(
                                out=psd[d][:, q * 512:(q + 1) * 512],
                                lhsT=w2t[ch][:, d * 128:(d + 1) * 128],
                                rhs=g_buf[:, (ch * nrows + r0) * 256:
                                          (ch * nrows + r0) * 256 + 512],
                                start=not stop, stop=stop,
                            )

                    mm_phase(0, 0, False)
                    mm_phase(0, 1, False)
                    mm_phase(1, 0, True)
                    for d in range(2):
                        if d == 1:
                            mm_phase(1, 1, True)
                        ost = opool.tile([128, 2048], BF16, tag="ostage")
                        if (drain_idx * ACT_DRAINS) % NSUB < ACT_DRAINS:
                            nc.scalar.copy(ost[:, :], psd[d][:, :])
                        else:
                            nc.vector.tensor_copy(ost[:, :], psd[d][:, :])
                        drain_idx += 1
                        nc.sync.dma_start(out=out[blk * 2 + d], in_=ost[:, :])
                row0 += nrows

    nc.finalize()
    return nc


def _get_nc():
    global _nc_cache
    if _nc_cache is None:
        _nc_cache = build_nc()
    return _nc_cache


def make_in_maps(x, query, W_pre, b_pre, W_emb, b_emb, W1, b1, W2, b2):
    x = np.asarray(x, np.float32)
    query = np.asarray(query, np.float32)
    W_pre = np.asarray(W_pre, np.float32)
    b_pre = np.asarray(b_pre, np.float32)
    W_emb = np.asarray(W_emb, np.float32)
    b_emb = np.asarray(b_emb, np.float32)
    W1 = np.asarray(W1, np.float32)
    b1 = np.asarray(b1, np.float32)
    W2 = np.asarray(W2, np.float32)

    xp = x.reshape(B * NI, DIN) @ W_pre + b_pre
    A = xp @ W1[:DH] + b1                       # [B*NI, DH]
    c = query.reshape(B * NO, DQ) @ W_emb + b_emb
    C2 = c @ W1[DH:]                            # [B*NO, DH]
    A = A.reshape(B, NI, DH)
    C2 = C2.reshape(B, NO, DH)

    w2b = np.ascontiguousarray(W2.astype(ml_dtypes.bfloat16))
    in_maps = []
    for k in range(NCORES):
        b = k // 2
        hh = k % 2
        in_maps.append({
            "c_t": np.ascontiguousarray(C2[b].T.astype(ml_dtypes.bfloat16)),
            "a_t": np.ascontiguousarray(A[b, hh * 128:(hh + 1) * 128, :].T),
            "w2": w2b,
        })
    return in_maps


def run_on_device(in_maps, trace=False):
    nc = _get_nc()
    return run_bass_kernel_spmd(nc, in_maps, core_ids=list(range(NCORES)), trace=trace)


def assemble(results, b2):
    out = np.empty((B, NI, NO, DOUT), np.float32)
    for k in range(NCORES):
        b = k // 2
        hh = k % 2
        # dev out: [blk*2+d, p, (r, o)] -> out[b, blk*8+r, o, d*128+p]
        dev = results[k]["out"].reshape(NSUB // 2, 2, 128, 8, 256)
        # axes (blk, d, p, r, o) -> (blk, r, o, d, p)
        dev = dev.transpose(0, 3, 4, 1, 2).reshape(RPC, NO, DOUT)
        out[b, hh * 128:(hh + 1) * 128] = dev.astype(np.float32)
    b2 = np.asarray(b2, np.float32)
    if np.any(b2):
        out += b2
    return out


def kernel(x, query, W_pre, b_pre, W_emb, b_emb, W1, b1, W2, b2):
    in_maps = make_in_maps(x, query, W_pre, b_pre, W_emb, b_emb, W1, b1, W2, b2)
    res = run_on_device(in_maps, trace=False)
    return assemble(res.results, b2)
